# revision 35
# baseline (speedup 1.0000x reference)
"""Trainium2 Bass kernel: LocalEmbeddingLayer (KNN -> gather -> 2-layer GELU MLP -> mean).

Full-input contract: kernel(**inputs) takes the unsharded inputs and returns the
full [B, N, P] output. Internally shards batch B=32 across 8 NeuronCores (pure
data parallel, 4 batch elements per core), runs one SPMD Bass program on all
cores, and concatenates the per-core outputs.

v2 design (replaces the ap_gather baseline, which spent ~33us/block of hidden
GPSIMD ucode time in the gather and serialized the whole pipeline behind it;
1827us -> 712us):

Per batch (amortized over its 8 row-blocks, prefetched half a batch ahead):
  - U^T token table: U[j] = W1[:64]^T f_j (256 fp16 per point) computed as 8
    PE matmuls (lhsT = featT chunk [64,128], rhs = W1a) -> PSUM -> ACT-copy to
    fp16 -> DMA to a DRAM table [1024, 256]. Gathering U instead of raw
    features removes the per-block layer-1 matmuls entirely.
  - Center term V+b1 is host-precomputed, k-broadcast, and shipped as the
    fp16 input `vbb` in the kernel's exact column order, so the on-device add
    is a flat packed-fp16 DVE tensor_tensor (stride-0 broadcast APs drop DVE
    to 1x; a host-materialized operand keeps the 2x mode).

Per 128-query block, 6-stage software pipeline (stages A, G, C1, C2, C3 at
lags 0/1/3/4/5 so PE/DVE/ACT/GPSIMD/DMA all overlap):
  - A: Tk[i,j] = 2*p_i.p_j - |p_j|^2 via K=16 bf16 hi/lo-split matmuls into
    PSUM f32 (~fp32 selection accuracy); a small matmul accumulates -200*I
    onto the diagonal block to mask self (finite in fp16 after rebase).
    ACT rebases to fp16: tkf = fp16(S*Tk + S*(c0 - r_i)) with host per-row
    bias; boundary-region fp16 quantum ~2.4e-4 in d^2 units (validated:
    0.76% boundary flips, ~5e-3 end-to-end rel err).
    Top-16 with DVE max8/find_index8/match_replace8 on the fp16 copy, 4 32x32
    DVE transposes to the [16, 128] descriptor-order layout, 2 SBUF->SBUF
    DMAs replicate it across all 8 Q7 index-partition groups.
  - G: GPSIMD dma_gather (SWDGE descriptors -> DMA engines) fetches the 2048
    U-rows from the DRAM table with transpose=True, landing h-on-partition as
    [128, chunk, half, 512] fp16, ready for layer 2. One gather instruction
    can only carry ~512 descriptors (16KB SWDGE ring), so the block's gather
    is 4 chunked 512-idx gathers. (Spreading chunks over num_swdge_queues>1
    silently corrupts on this runtime - keep a single queue.)
  - C1: h1 = U_g + vbb on DVE (packed fp16 2x), gelu1 on ACT [128, 4096].
  - C2: layer-2 on PE (fp16, 2xLDWEIGHTS + 8x512-col MMs into two ping-pong
    [128,1024] PSUM tiles), gelu2+bias on ACT writing fp16.
  - C3: mean-over-k as a packed-fp16 TT add tree on DVE (1024/512/256/128),
    fp16 [P, i] block DMA'd out (host transposes and applies the 1/16).
"""

import numpy as np

B, N, DPOS, F, P, K = 32, 1024, 3, 64, 128, 16
NCORES = 8
BL = B // NCORES          # batches per core
NBLK = N // 128           # row blocks per batch
NEG = -200.0              # self-mask diag value (finite in fp16 after rebase)
SCALE = 256.0             # topk rebase scale
C0 = 1.0                  # topk rebase center
MREP = -60000.0           # match_replace fill, below every real/masked score


def build_program(n_b=BL, n_blk=NBLK):
    import concourse.bacc as bacc
    import concourse.mybir as mybir
    from concourse.tile import TileContext

    f32 = mybir.dt.float32
    f16 = mybir.dt.float16
    bf16 = mybir.dt.bfloat16
    u16 = mybir.dt.uint16
    i16 = mybir.dt.int16
    AF = mybir.ActivationFunctionType

    nc = bacc.Bacc(
        "TRN2", target_bir_lowering=False,
        num_swdge_queues=1, dynamic_dma_scratch_size=16384,
    )

    featf_d = nc.dram_tensor("featf", [n_b, 64, N], f16, kind="ExternalInput")
    ab_d = nc.dram_tensor("ab", [n_b, 2, 16, N], bf16, kind="ExternalInput")
    rb_d = nc.dram_tensor("rb", [n_b, 128, NBLK], f32, kind="ExternalInput")
    w1a_d = nc.dram_tensor("w1a", [64, 256], f16, kind="ExternalInput")
    vbb_d = nc.dram_tensor("vbb", [n_b, NBLK, 128, 4096], f16, kind="ExternalInput")
    w2_d = nc.dram_tensor("w2", [128, 256], f16, kind="ExternalInput")
    b2_d = nc.dram_tensor("b2", [128, 1], f32, kind="ExternalInput")
    cbf_d = nc.dram_tensor("cbf", [128, 256], bf16, kind="ExternalInput")
    out_d = nc.dram_tensor("out", [n_b, n_blk, 128, 128], f16, kind="ExternalOutput")

    total = n_b * n_blk

    with TileContext(nc) as tc:
        with (
            tc.tile_pool(name="const", bufs=1) as cpool,
            tc.tile_pool(name="batch", bufs=2) as bpool,
            tc.tile_pool(name="dram", bufs=2, space="DRAM") as dpool,
            tc.tile_pool(name="knn", bufs=2) as kpool,
            tc.tile_pool(name="gath", bufs=4) as gpool,
            tc.tile_pool(name="mlp", bufs=3) as mpool,
            tc.tile_pool(name="ps_a", bufs=2, space="PSUM") as ptk,
            tc.tile_pool(name="ps_b", bufs=2, space="PSUM") as pl2,
        ):
            w1a_sb = cpool.tile([64, 256], f16)
            nc.sync.dma_start(w1a_sb[:], w1a_d[:])
            w2_sb = cpool.tile([128, 256], f16)
            nc.sync.dma_start(w2_sb[:], w2_d[:])
            b2_sb = cpool.tile([128, 1], f32)
            nc.sync.dma_start(b2_sb[:], b2_d[:])
            cbf_sb = cpool.tile([128, 256], bf16)   # cols 0:128 I, 128:256 NEG*I
            nc.sync.dma_start(cbf_sb[:], cbf_d[:])

            bh = {}   # per-batch handles
            sh = {}   # per-block handles

            def prep(b):
                featf = bpool.tile([64, N], f16, tag="featf")
                nc.sync.dma_start(featf[:], featf_d[b])
                ab = bpool.tile([16, 2 * N], bf16, tag="ab")
                nc.sync.dma_start(
                    ab[:].rearrange("d (x n) -> d x n", x=2),
                    ab_d[b].rearrange("x d n -> d x n"),
                )
                rb = bpool.tile([128, NBLK], f32, tag="rb")
                nc.sync.dma_start(rb[:], rb_d[b])

                # U^T token table: chunk c holds tokens j = c*128 + p.
                utsb = bpool.tile([128, 2048], f16, tag="utsb")
                for half in range(2):
                    up = pl2.tile([128, 1024], f32, tag="l2")
                    for cc in range(4):
                        c = half * 4 + cc
                        nc.tensor.matmul(
                            up[:, cc * 256:(cc + 1) * 256],
                            featf[:, c * 128:(c + 1) * 128],
                            w1a_sb[:],
                            start=True, stop=True,
                        )
                    nc.scalar.activation(
                        utsb[:, half * 1024:(half + 1) * 1024], up[:], AF.Copy
                    )
                utd = dpool.tile([N, 256], f16, tag="ut")
                nc.sync.dma_start(
                    utd[:].rearrange("(c p) h -> p c h", p=128),
                    utsb[:].rearrange("p (c h) -> p c h", c=8),
                )
                bh[b] = (ab, rb, utd)

            def stageA(s):
                b, blk = divmod(s, n_blk)
                ab, rb, _ = bh[b]
                tk = ptk.tile([128, 1024], f32, tag="tk")
                lhsA = ab[:, blk * 128:(blk + 1) * 128]
                for h in range(2):
                    nc.tensor.matmul(
                        tk[:, h * 512:(h + 1) * 512],
                        lhsA,
                        ab[:, N + h * 512:N + (h + 1) * 512],
                        start=True, stop=True,
                    )
                nc.tensor.matmul(
                    tk[:, blk * 128:(blk + 1) * 128],
                    cbf_sb[:, 0:128],
                    cbf_sb[:, 128:256],
                    start=False, stop=True,
                    skip_group_check=True,
                )

                # rebase to fp16 on the scalar engine:
                # tkf = fp16(SCALE*tk + SCALE*(c0 - r_i))
                tkf = kpool.tile([128, 1024], f16, tag="tkf")
                nc.scalar.activation(
                    tkf[:], tk[:], AF.Identity,
                    bias=rb[:, blk:blk + 1], scale=SCALE,
                )

                vals = kpool.tile([128, 16], f16, tag="vals")
                idxp = kpool.tile([128, 32], u16, tag="idxp")
                nc.vector.max(vals[:, 0:8], tkf[:])
                nc.vector.max_index(idxp[:, 0:8], vals[:, 0:8], tkf[:])
                nc.vector.match_replace(tkf[:], vals[:, 0:8], tkf[:], MREP)
                nc.vector.max(vals[:, 8:16], tkf[:])
                nc.vector.max_index(idxp[:, 8:16], vals[:, 8:16], tkf[:])
                nc.vector.tensor_copy(idxp[:, 16:32], idxp[:, 0:16])

                # [128,32] -> rows 0:32 of idxR (rows 0:16 valid, 16:32 dup)
                idxR = kpool.tile([128, 128], u16, tag="idxR")
                for t4 in range(4):
                    nc.vector.transpose(
                        idxR[0:32, 32 * t4:32 * (t4 + 1)],
                        idxp[32 * t4:32 * (t4 + 1), 0:32],
                    )
                # replicate across the 8 16-partition index groups
                nc.sync.dma_start(idxR[32:64, :], idxR[0:32, :])
                nc.sync.dma_start(idxR[64:128, :], idxR[0:64, :])
                sh[s] = {"idxR": idxR}

            def stageG(s):
                b, blk = divmod(s, n_blk)
                _, _, utd = bh[b]
                idxR = sh[s]["idxR"]
                # chunk-major: [p, chunk(4), h-half(2), 512]; each 512-idx
                # gather writes one contiguous chunk slab.  SWDGE descriptor
                # ring caps one gather at ~512 descriptors; spread the four
                # chunks across the four SWDGE queues so their DMA transfers
                # run on different engines.
                nb = gpool.tile([128, 4, 2, 512], f16, tag="nb")
                for c in range(4):
                    nc.gpsimd.dma_gather(
                        nb[:, c],
                        utd[:],
                        idxR[:, c * 32:(c + 1) * 32].bitcast(i16),
                        512, 512, 256, transpose=True,
                    )
                sh[s]["nb"] = nb
                # prefetch the host-prebroadcast center term (V+b1, already in
                # the same (chunk, half, il, k) column order as nb)
                vbB = gpool.tile([128, 4096], f16, tag="vbB")
                nc.sync.dma_start(vbB[:], vbb_d[b, blk])
                sh[s]["vbB"] = vbB

            def stageC1(s):
                nb = sh[s]["nb"]
                vbB = sh[s]["vbB"]
                h1 = mpool.tile([128, 4096], f16, tag="h1")
                nc.vector.tensor_tensor(
                    h1[:], nb[:].rearrange("p c h g -> p (c h g)"), vbB[:],
                    mybir.AluOpType.add,
                )
                g1 = mpool.tile([128, 4096], f16, tag="g1")
                nc.scalar.activation(g1[:], h1[:], AF.Gelu)
                sh[s]["g1"] = g1

            def stageC2(s):
                g1 = sh[s]["g1"]
                g2 = mpool.tile([128, 2048], f16, tag="g2")
                p2a = pl2.tile([128, 1024], f32, tag="l2")
                p2b = pl2.tile([128, 1024], f32, tag="l2")
                for h in range(2):
                    for c in range(4):
                        pdst = p2a if c < 2 else p2b
                        nc.tensor.matmul(
                            pdst[:, (c % 2) * 512:(c % 2 + 1) * 512],
                            w2_sb[:, h * 128:(h + 1) * 128],
                            g1[:, c * 1024 + h * 512:c * 1024 + (h + 1) * 512],
                            start=(h == 0), stop=(h == 1),
                        )
                nc.scalar.activation(
                    g2[:, 0:1024], p2a[:], AF.Gelu, bias=b2_sb[:, 0:1]
                )
                nc.scalar.activation(
                    g2[:, 1024:2048], p2b[:], AF.Gelu, bias=b2_sb[:, 0:1]
                )
                sh[s]["g2"] = g2

            def stageC3(s):
                b, blk = divmod(s, n_blk)
                g2 = sh[s]["g2"]
                t1 = mpool.tile([128, 1024], f16, tag="t1")
                nc.vector.tensor_tensor(
                    t1[:].rearrange("p (i k) -> p i k", k=8),
                    g2[:].rearrange("p (i k) -> p i k", k=16)[:, :, 0:8],
                    g2[:].rearrange("p (i k) -> p i k", k=16)[:, :, 8:16],
                    mybir.AluOpType.add,
                )
                t2 = mpool.tile([128, 512], f16, tag="t2")
                nc.vector.tensor_tensor(
                    t2[:].rearrange("p (i k) -> p i k", k=4),
                    t1[:].rearrange("p (i k) -> p i k", k=8)[:, :, 0:4],
                    t1[:].rearrange("p (i k) -> p i k", k=8)[:, :, 4:8],
                    mybir.AluOpType.add,
                )
                t3 = mpool.tile([128, 256], f16, tag="t3")
                nc.vector.tensor_tensor(
                    t3[:].rearrange("p (i k) -> p i k", k=2),
                    t2[:].rearrange("p (i k) -> p i k", k=4)[:, :, 0:2],
                    t2[:].rearrange("p (i k) -> p i k", k=4)[:, :, 2:4],
                    mybir.AluOpType.add,
                )
                red = mpool.tile([128, 128], f16, tag="red")
                nc.vector.tensor_tensor(
                    red[:].rearrange("p (i k) -> p i k", k=1),
                    t3[:].rearrange("p (i k) -> p i k", k=2)[:, :, 0:1],
                    t3[:].rearrange("p (i k) -> p i k", k=2)[:, :, 1:2],
                    mybir.AluOpType.add,
                )
                nc.sync.dma_start(out_d[b, blk], red[:])
                del sh[s]

            prep(0)
            for it in range(total + 4):
                if 0 <= it - 1 < total:
                    stageG(it - 1)
                if 0 <= it - 3 < total:
                    stageC1(it - 3)
                if it < total:
                    stageA(it)
                if it < total and it % n_blk == 4 and (it // n_blk + 1) < n_b:
                    prep(it // n_blk + 1)
                if 0 <= it - 3 < total:
                    stageC2(it - 3)
                if 0 <= it - 4 < total:
                    stageC3(it - 4)

    nc.compile()
    return nc


def prep_core_inputs(points, features, W1, b1, W2, b2, core):
    """Host-side packing of one core's inputs (batches core*BL .. core*BL+BL)."""
    import ml_dtypes
    bf = ml_dtypes.bfloat16
    sl = slice(core * BL, (core + 1) * BL)
    pts = points[sl]           # [BL, N, 3]
    fts = features[sl]         # [BL, N, F]

    featf = np.ascontiguousarray(fts.transpose(0, 2, 1)).astype(np.float16)

    r = (pts.astype(np.float64) ** 2).sum(-1).astype(np.float32)  # [BL, N]
    p_hi = pts.astype(bf).astype(np.float32)
    p_lo = (pts - p_hi).astype(bf).astype(np.float32)
    r_hi = r.astype(bf).astype(np.float32)
    r_lo = (r - r_hi).astype(bf).astype(np.float32)

    ab = np.zeros((BL, 2, 16, N), np.float32)
    # lhs rows (A) pair with rhs rows (B); Tk = 2 p_i . p_j - r_j
    ab[:, 0, 0:3] = 2.0 * p_hi.transpose(0, 2, 1)
    ab[:, 0, 3:6] = 2.0 * p_lo.transpose(0, 2, 1)
    ab[:, 0, 6:9] = 2.0 * p_hi.transpose(0, 2, 1)
    ab[:, 0, 9] = -1.0
    ab[:, 0, 10] = -1.0
    ab[:, 1, 0:3] = p_hi.transpose(0, 2, 1)
    ab[:, 1, 3:6] = p_hi.transpose(0, 2, 1)
    ab[:, 1, 6:9] = p_lo.transpose(0, 2, 1)
    ab[:, 1, 9] = r_hi
    ab[:, 1, 10] = r_lo
    ab = ab.astype(bf)

    rb = SCALE * (C0 - r.reshape(BL, NBLK, 128).transpose(0, 2, 1))
    rb = np.ascontiguousarray(rb).astype(np.float32)

    w1a = W1[0:64].astype(np.float16)
    w2p = np.empty((128, 256), np.float16)
    w2p[:, 0:128] = W2[0:128]
    w2p[:, 128:256] = W2[128:256]
    b2p = np.ascontiguousarray(b2.reshape(128, 1)).astype(np.float32)

    eye = np.eye(128, dtype=np.float32)
    cbf = np.concatenate([eye, NEG * eye], axis=1).astype(bf)

    # host-prebroadcast center term V + b1 in the kernel's (chunk, half,
    # il, k) column order: vbb[b, blk, p, c*1024 + h*512 + il*16 + k]
    #   = V[h*128+p, blk*128 + c*32 + il] + b1[h*128+p]
    Vh = (fts @ (W1[64:128] - W1[0:64]) + b1).astype(np.float16)  # [BL, N, 256]
    Vr = Vh.reshape(BL, NBLK, 4, 32, 2, 128)            # [b, blk, c, il, h, p]
    vbb = np.broadcast_to(
        Vr.transpose(0, 1, 5, 2, 4, 3)[..., None],      # [b, blk, p, c, h, il, 1]
        (BL, NBLK, 128, 4, 2, 32, K),
    ).reshape(BL, NBLK, 128, 4096)

    return {
        "featf": featf,
        "ab": np.ascontiguousarray(ab),
        "rb": rb,
        "w1a": np.ascontiguousarray(w1a),
        "vbb": np.ascontiguousarray(vbb),
        "w2": w2p, "b2": b2p,
        "cbf": np.ascontiguousarray(cbf),
    }


_CACHED = {}


def kernel(points, features, W1, b1, W2, b2):
    from concourse import bass_utils

    points = np.asarray(points, np.float32)
    features = np.asarray(features, np.float32)
    W1 = np.asarray(W1, np.float32)
    b1 = np.asarray(b1, np.float32)
    W2 = np.asarray(W2, np.float32)
    b2 = np.asarray(b2, np.float32)

    if "nc" not in _CACHED:
        _CACHED["nc"] = build_program()
    nc = _CACHED["nc"]

    in_maps = [
        prep_core_inputs(points, features, W1, b1, W2, b2, c)
        for c in range(NCORES)
    ]
    res = bass_utils.run_bass_kernel_spmd(
        nc, in_maps, core_ids=list(range(NCORES))
    )
    outs = []
    for c in range(NCORES):
        o = res.results[c]["out"]          # [BL, NBLK, 128, 128] = [b, blk, P, i] f16
        o = o.astype(np.float32) * (1.0 / K)
        outs.append(o.transpose(0, 1, 3, 2).reshape(BL, N, P))
    return np.concatenate(outs, axis=0)
